# revision 1
# baseline (speedup 1.0000x reference)
"""ConfidenceGate Trainium2 kernel (8 NeuronCores, SPMD).

Problem recap (shapes hardcoded from the spec):
  x:      (4, 512, 256, 7, 7) f32
  prev_x: (4, 512, 256, 7, 7) f32
  match:  (4, 512, 513) f32
  + tiny proj/LN/MLP params.
Reference returns c[0] -> (512, 1): only batch 0 contributes to the output.

Strategy (v2):
  * Only batch 0 is computed; data-parallel over M=512 rows: 8 cores x 64.
  * top1 = argmax(match[0,:,:512]) on host; prev rows pre-gathered per shard.
  * Host stores both big streams CHANNEL-MAJOR fp16, spatial padded 49->50:
    dram[c_half, roi*50 + s].  The on-device grouped reduce (DVE, fp16)
    directly produces the proj-matmul lhsT (c, roi) -- no transposes or
    deinterleaves anywhere in the streaming path.  1/49 is folded into the
    weights; the rhs carries an extra NEGATED column-mean column so the
    psum holds [v | -mu] and centering is one ACT bias-add.
  * All transcendentals use ONE activation table set (natural_log + exp):
    1/sqrt(q) = exp(-0.5 ln q), sigmoid(z) = 1/(1+exp(-z)) via DVE recip.
  * match is fp16 too; stats spread over gpsimd (TT/TS only -- no STT
    there) + ACT, keeping DVE for pooling reduces + the few accum ops.
  * MLP hidden state pre-accumulated over the 4 early features during
    streaming (ACT per-partition-scale copies + Pool adds); the tail only
    folds in cos_sim on DVE.
"""

import sys

if "/opt/trn_rl_repo" not in sys.path:
    sys.path.insert(0, "/opt/trn_rl_repo")

import numpy as np

B, M, N, C, G = 4, 512, 512, 256, 7
S = G * G                      # 49 spatial positions
SP = 50                        # padded spatial (4B-aligned fp16 groups)
PP, HH = 32, 32                # proj dim, MLP hidden
NCORES = 8
MS = M // NCORES               # 64 rows per core
CH = C // 2                    # 128 channels per half = partition dim
COLS = MS * SP                 # 3200 cols per half
EPS = 1e-9
LN_EPS = 1e-5
NEG = -60000.0                 # fp16-safe "-inf" for second-max masking

# roi sub-chunk splits: (which, half, roi_off, roi_len); v first, x last,
# with the final sync-ring chunk tapered so the last reduce is small.
CHUNKS = [
    ("v", 0, 0, 32), ("v", 0, 32, 32),
    ("v", 1, 0, 32), ("v", 1, 32, 32),
    ("x", 0, 0, 32), ("x", 0, 32, 32),
    ("x", 1, 0, 16), ("x", 1, 16, 16),
]
# x half-1 rois 32..63 ride the scalar ring (issued after mt/aux, so they
# land mid-stream and their reduces run early)
CHUNKS_SC = [("x", 1, 32, 16), ("x", 1, 48, 16)]

# auxf (f32, 64 partitions) column layout
A_PB = 0          # psum preload [proj_b | -mean(proj_b)] (64, 33)
A_LG = 33         # ln_g replicated (64, 32)
A_LB = 65         # ln_b replicated (64, 32)
A_W1 = 97         # w1 replicated: block f at [97+32f : 129+32f), f=0..4
A_B1 = 257        # b1 replicated (64, 32)
A_W2 = 289        # w2[0] replicated (64, 32)
A_NB2 = 321       # -b2 replicated (64, 1)
A_COLS = 322

_CACHE = {}


def _build():
    import concourse.bacc as bacc
    import concourse.tile as tile
    import concourse.mybir as mybir

    dt = mybir.dt
    Alu = mybir.AluOpType
    Act = mybir.ActivationFunctionType
    Ax = mybir.AxisListType
    f32 = dt.float32
    f16 = dt.float16

    nc = bacc.Bacc("TRN2", target_bir_lowering=False, debug=False)

    xs_d = nc.dram_tensor("xs", [CH, 2 * COLS], f16, kind="ExternalInput")
    pv_d = nc.dram_tensor("pv", [CH, 2 * COLS], f16, kind="ExternalInput")
    mt_d = nc.dram_tensor("mt", [MS, N + 1], f16, kind="ExternalInput")
    a16_d = nc.dram_tensor("a16", [CH, 66], f16, kind="ExternalInput")
    axf_d = nc.dram_tensor("axf", [MS, A_COLS], f32, kind="ExternalInput")
    out_d = nc.dram_tensor("out", [MS, 1], f32, kind="ExternalOutput")

    with tile.TileContext(nc) as tc, nc.allow_low_precision(
        reason="fp16 pooling sums; |sum|<=~30, rel err ~1e-3 vs 2e-2 gate"
    ):
        with (
            tc.tile_pool(name="persist", bufs=1) as per,
            tc.tile_pool(name="chunks", bufs=1) as big,
            tc.tile_pool(name="scratch", bufs=1) as scr,
            tc.tile_pool(name="psum", bufs=1, space="PSUM") as psp,
        ):
            # ---- small loads on the scalar (ACT) HWDGE ring ----
            mt = per.tile([MS, N + 1], f16)
            nc.scalar.dma_start(out=mt[:], in_=mt_d[:])
            a16 = per.tile([CH, 66], f16)
            nc.scalar.dma_start(out=a16[:], in_=a16_d[:])
            axf = per.tile([MS, A_COLS], f32)
            nc.scalar.dma_start(out=axf[:], in_=axf_d[:])

            # ---- constants + hoist the (single) ACT table load to t=0 ----
            e9 = per.tile([MS, 1], f32)
            nc.gpsimd.memset(e9[:], EPS)
            eln = per.tile([MS, 1], f32)
            nc.gpsimd.memset(eln[:], LN_EPS)
            dmy = per.tile([1, 1], f32)
            nc.gpsimd.memset(dmy[:], 1.0)
            pre = scr.tile([1, 2], f32, tag="pre")
            nc.scalar.activation(pre[:, 0:1], dmy[:], Act.Ln, bias=e9[0:1, 0:1])
            nc.scalar.activation(pre[:, 1:2], dmy[:], Act.Exp)

            # ---- big chunked loads ----
            src = {"x": xs_d, "v": pv_d}
            ctiles = {}
            for w, h, ro, rl in CHUNKS:
                t = big.tile([CH, rl * SP], f16, tag=f"ch_{w}{h}_{ro}",
                             name=f"ch_{w}{h}_{ro}")
                co = h * COLS + ro * SP
                nc.sync.dma_start(out=t[:], in_=src[w][:, co:co + rl * SP])
                ctiles[(w, h, ro)] = t
            for w, h, ro, rl in CHUNKS_SC:
                t = big.tile([CH, rl * SP], f16, tag=f"ch_{w}{h}_{ro}",
                             name=f"ch_{w}{h}_{ro}")
                co = h * COLS + ro * SP
                nc.scalar.dma_start(out=t[:], in_=src[w][:, co:co + rl * SP])
                ctiles[(w, h, ro)] = t

            real = mt[:, 0:N]
            pd = mt[:, N:N + 1]

            # ---- match stats (overlapped with streaming) ----
            rmass = per.tile([MS, 1], f32)
            jr = scr.tile([MS, N], f16, tag="jk")
            nc.scalar.activation(jr[:], real, Act.Copy, accum_out=rmass[:])
            lnr = per.tile([MS, N], f16)
            nc.scalar.activation(lnr[:], real, Act.Ln, bias=e9[:])
            # p_max [DVE, fp16 2x]
            f_pmax = per.tile([MS, 1], f32)
            nc.vector.reduce_max(f_pmax[:], real, axis=Ax.X)
            # second max via mask-out [gpsimd] + DVE reduce
            eqm = scr.tile([MS, N], f16, tag="jk3")
            nc.gpsimd.tensor_scalar(eqm[:], real, f_pmax[:], None,
                                    op0=Alu.is_equal)
            nmsk = scr.tile([MS, N], f16, tag="jk4")
            nc.gpsimd.tensor_scalar(nmsk[:], eqm[:], NEG, None, op0=Alu.mult)
            msk = scr.tile([MS, N], f16, tag="jk5")
            nc.gpsimd.tensor_tensor(msk[:], nmsk[:], real, op=Alu.add)
            m2 = per.tile([MS, 1], f32)
            nc.vector.reduce_max(m2[:], msk[:], axis=Ax.X)
            f_gap = per.tile([MS, 1], f32)
            nc.gpsimd.tensor_tensor(f_gap[:], f_pmax[:], m2[:], op=Alu.subtract)
            # entropy: f_ent = sum(real * ln(real+eps)) = -ent
            je = scr.tile([MS, N], f16, tag="jk2")
            nc.gpsimd.tensor_tensor(je[:], real, lnr[:], op=Alu.mult)
            f_ent = per.tile([MS, 1], f32)
            je2 = scr.tile([MS, N], f16, tag="jk6")
            nc.scalar.activation(je2[:], je[:], Act.Copy, accum_out=f_ent[:])
            # feat0 = 1 - p_dummy [DVE]
            f_pd = per.tile([MS, 1], f32)
            nc.vector.tensor_scalar(f_pd[:], pd, -1.0, 1.0, op0=Alu.mult,
                                    op1=Alu.add)
            # masks [gpsimd]
            hr9 = per.tile([MS, 1], f32)
            nc.gpsimd.tensor_scalar(hr9[:], rmass[:], EPS, None, op0=Alu.is_gt)
            hr6 = per.tile([MS, 1], f32)
            nc.gpsimd.tensor_scalar(hr6[:], rmass[:], 1e-6, None, op0=Alu.is_gt)

            # ---- MLP hidden pre-accumulation over the 4 early features ----
            # tmp_f = w1[:,f]*feat_f  (ACT per-partition scale), summed on Pool
            tmps = []
            for i, fap in enumerate((f_pd, f_pmax, f_gap, f_ent)):
                t = scr.tile([MS, HH], f32, tag=f"tmpf{i}")
                nc.scalar.activation(
                    t[:], axf[:, A_W1 + i * HH:A_W1 + (i + 1) * HH],
                    Act.Copy, scale=fap[:])
                tmps.append(t)
            hAc = per.tile([MS, HH], f32, tag="hA")
            nc.gpsimd.tensor_tensor(hAc[:], tmps[0][:], axf[:, A_B1:A_B1 + HH],
                                    op=Alu.add)
            hBc = per.tile([MS, HH], f32, tag="hB")
            nc.gpsimd.tensor_tensor(hBc[:], hAc[:], tmps[1][:], op=Alu.add)
            hCc = per.tile([MS, HH], f32, tag="hC")
            nc.gpsimd.tensor_tensor(hCc[:], hBc[:], tmps[2][:], op=Alu.add)
            hDc = per.tile([MS, HH], f32, tag="hD")
            nc.gpsimd.tensor_tensor(hDc[:], hCc[:], tmps[3][:], op=Alu.add)

            # ---- proj psum tiles preloaded with [proj_b | -mean(proj_b)] ----
            vps = {}
            for w in ("x", "v"):
                t = psp.tile([MS, PP + 1], f32, tag=f"vps_{w}", name=f"vps_{w}")
                nc.scalar.activation(t[:], axf[:, A_PB:A_PB + PP + 1], Act.Copy)
                vps[w] = t

            # ---- streaming: grouped pool reduce -> P_h; matmul per half ----
            P_t = {}
            for w in ("x", "v"):
                for h in (0, 1):
                    P_t[(w, h)] = per.tile([CH, MS], f16, tag=f"P_{w}{h}",
                                           name=f"P_{w}{h}")
            nchunks = {}
            for w, h, ro, rl in CHUNKS + CHUNKS_SC:
                nchunks[(w, h)] = nchunks.get((w, h), 0) + 1
            done = {}
            for w, h, ro, rl in CHUNKS + CHUNKS_SC:
                ct = ctiles[(w, h, ro)]
                P = P_t[(w, h)]
                nc.vector.reduce_sum(
                    P[:, ro:ro + rl],
                    ct[:].rearrange("p (r s) -> p r s", s=SP), axis=Ax.X)
                done[(w, h)] = done.get((w, h), 0) + 1
                if done[(w, h)] == nchunks[(w, h)]:
                    nc.tensor.matmul(
                        vps[w][:], P[:], a16[:, h * 33:(h + 1) * 33],
                        start=False, stop=(h == 1), skip_group_check=True)

            # ---- layernorm chains (v overlapped with x streaming) ----
            ys = {}
            for w in ("v", "x"):
                vp = vps[w]
                # center: v + (-mu)  (DVE per-partition scalar add from PSUM)
                ctr = per.tile([MS, PP], f32, tag=f"ctr_{w}")
                nc.vector.tensor_scalar(ctr[:], vp[:, 0:PP], vp[:, PP:PP + 1],
                                        None, op0=Alu.add)
                jv = scr.tile([MS, PP], f32, tag=f"jv_{w}")
                vsum = per.tile([MS, 1], f32, tag=f"vs_{w}")
                nc.scalar.activation(jv[:], ctr[:], Act.Square,
                                     accum_out=vsum[:])
                lnv = scr.tile([MS, 1], f32, tag=f"lnv_{w}")
                nc.scalar.activation(lnv[:], vsum[:], Act.Ln, scale=1.0 / PP,
                                     bias=eln[:])
                rs = per.tile([MS, 1], f32, tag=f"rs_{w}")
                nc.scalar.activation(rs[:], lnv[:], Act.Exp, scale=-0.5)
                # g*rstd on Pool, then y = ctr*(g*rstd) + b on Pool
                gr = scr.tile([MS, PP], f32, tag=f"gr_{w}")
                nc.gpsimd.tensor_scalar(gr[:], axf[:, A_LG:A_LG + PP], rs[:],
                                        None, op0=Alu.mult)
                yg = scr.tile([MS, PP], f32, tag=f"yg_{w}")
                nc.gpsimd.tensor_tensor(yg[:], ctr[:], gr[:], op=Alu.mult)
                y = per.tile([MS, PP], f32, tag=f"y_{w}")
                nc.gpsimd.tensor_tensor(y[:], yg[:], axf[:, A_LB:A_LB + PP],
                                        op=Alu.add)
                ys[w] = y

            # ---- cosine similarity ----
            yx, yv = ys["x"], ys["v"]
            nrm = per.tile([MS, 2], f32)
            jn = scr.tile([MS, PP], f32, tag="jn")
            nc.vector.scalar_tensor_tensor(
                jn[:], yv[:], 1.0, yv[:], op0=Alu.mult, op1=Alu.mult,
                accum_out=nrm[:, 1:2])
            jn2 = scr.tile([MS, PP], f32, tag="jn2")
            nc.vector.scalar_tensor_tensor(
                jn2[:], yx[:], 1.0, yx[:], op0=Alu.mult, op1=Alu.mult,
                accum_out=nrm[:, 0:1])
            dot = per.tile([MS, 1], f32)
            jn3 = scr.tile([MS, PP], f32, tag="jn3")
            nc.vector.scalar_tensor_tensor(
                jn3[:], yx[:], 1.0, yv[:], op0=Alu.mult, op1=Alu.mult,
                accum_out=dot[:])
            q = per.tile([MS, 1], f32)
            nc.vector.tensor_tensor(q[:], nrm[:, 0:1], nrm[:, 1:2], op=Alu.mult)
            lnq = scr.tile([MS, 1], f32, tag="lnq")
            nc.scalar.activation(lnq[:], q[:], Act.Ln)
            rq = per.tile([MS, 1], f32)
            nc.scalar.activation(rq[:], lnq[:], Act.Exp, scale=-0.5)
            f_cos = per.tile([MS, 1], f32)
            nc.vector.scalar_tensor_tensor(
                f_cos[:], dot[:], rq[:], hr9[:], op0=Alu.mult, op1=Alu.mult)

            # ---- finish MLP on DVE ----
            hE = per.tile([MS, HH], f32, tag="hE")
            nc.vector.scalar_tensor_tensor(
                hE[:], axf[:, A_W1 + 4 * HH:A_W1 + 5 * HH], f_cos[:], hDc[:],
                op0=Alu.mult, op1=Alu.add)
            hR = per.tile([MS, HH], f32, tag="hR")
            nc.vector.tensor_scalar(hR[:], hE[:], 0.0, None, op0=Alu.max)
            logit = per.tile([MS, 1], f32)
            jl = scr.tile([MS, HH], f32, tag="jl")
            nc.vector.scalar_tensor_tensor(
                jl[:], hR[:], 1.0, axf[:, A_W2:A_W2 + HH], op0=Alu.mult,
                op1=Alu.mult, accum_out=logit[:])
            # sigmoid = 1/(1+exp(-z));  exp(-logit - b2)
            ez = per.tile([MS, 1], f32)
            nc.scalar.activation(ez[:], logit[:], Act.Exp, scale=-1.0,
                                 bias=axf[:, A_NB2:A_NB2 + 1])
            den = per.tile([MS, 1], f32)
            nc.vector.tensor_scalar(den[:], ez[:], 1.0, None, op0=Alu.add)
            sg = per.tile([MS, 1], f32)
            nc.vector.reciprocal(sg[:], den[:])
            gt = per.tile([MS, 1], f32)
            nc.vector.scalar_tensor_tensor(
                gt[:], sg[:], 0.999, hr6[:], op0=Alu.min, op1=Alu.mult)
            res = per.tile([MS, 1], f32)
            nc.vector.tensor_scalar(res[:], gt[:], 0.001, None, op0=Alu.max)
            nc.sync.dma_start(out=out_d[:], in_=res[:])

    nc.finalize()
    return nc


def _get_nc():
    if "nc" not in _CACHE:
        _CACHE["nc"] = _build()
    return _CACHE["nc"]


def _pack_stream(rows_f32):
    """(64, 256*49) f32 -> (128, 6400) f16 channel-major padded layout."""
    f16 = np.float16
    t = rows_f32.reshape(MS, C, S).transpose(1, 0, 2)      # (256, 64, 49)
    buf = np.zeros((C, MS, SP), dtype=f16)
    buf[:, :, :S] = t
    out = np.empty((CH, 2 * COLS), dtype=f16)
    out[:, :COLS] = buf[0:CH].reshape(CH, COLS)
    out[:, COLS:] = buf[CH:C].reshape(CH, COLS)
    return out


def make_in_maps(x, prev_x, match, proj_w, proj_b, ln_g, ln_b, w1, b1, w2, b2):
    f32 = np.float32
    f16 = np.float16
    x0 = np.asarray(x[0], dtype=f32).reshape(M, C, S)
    p0 = np.asarray(prev_x[0], dtype=f32).reshape(N, C, S)
    mt0 = np.ascontiguousarray(np.asarray(match[0], dtype=f32))
    real0 = mt0[:, :N]
    rm = real0.sum(axis=1)
    top1 = np.where(rm > EPS, np.argmax(real0, axis=1), 0)

    proj_w = np.asarray(proj_w, dtype=f32)   # (32, 256)
    proj_b = np.asarray(proj_b, dtype=f32)
    # a16 per half h: [c, 0:32] = proj_w[:, 128h+c].T/49 ; col 32 = -rowmean/32
    a16 = np.zeros((CH, 66), dtype=f16)
    for h in (0, 1):
        blk = proj_w[:, h * CH:(h + 1) * CH].T / S       # (128, 32)
        a16[:, h * 33:h * 33 + PP] = blk
        a16[:, h * 33 + PP] = -blk.mean(axis=1)
    axf = np.zeros((MS, A_COLS), dtype=f32)
    axf[:, A_PB:A_PB + PP] = proj_b
    axf[:, A_PB + PP] = -proj_b.mean()
    axf[:, A_LG:A_LG + PP] = np.asarray(ln_g, dtype=f32)
    axf[:, A_LB:A_LB + PP] = np.asarray(ln_b, dtype=f32)
    w1 = np.asarray(w1, dtype=f32)           # (32, 5)
    for f in range(5):
        axf[:, A_W1 + f * HH:A_W1 + (f + 1) * HH] = w1[:, f]
    axf[:, A_B1:A_B1 + HH] = np.asarray(b1, dtype=f32)
    axf[:, A_W2:A_W2 + HH] = np.asarray(w2, dtype=f32)[0]
    axf[:, A_NB2] = -np.asarray(b2, dtype=f32)[0]

    in_maps = []
    for i in range(NCORES):
        lo, hi = i * MS, (i + 1) * MS
        in_maps.append({
            "xs": _pack_stream(x0[lo:hi]),
            "pv": _pack_stream(p0[top1[lo:hi]]),
            "mt": np.ascontiguousarray(mt0[lo:hi]).astype(f16),
            "a16": a16, "axf": axf,
        })
    return in_maps


def run(in_maps, trace=False):
    from concourse.bass_utils import run_bass_kernel_spmd
    res = run_bass_kernel_spmd(_get_nc(), in_maps, list(range(NCORES)), trace=trace)
    out = np.concatenate(
        [res.results[i]["out"].reshape(MS, 1) for i in range(NCORES)], axis=0)
    return out.astype(np.float32), res


def kernel(x, prev_x, match, proj_w, proj_b, ln_g, ln_b, w1, b1, w2, b2):
    in_maps = make_in_maps(x, prev_x, match, proj_w, proj_b, ln_g, ln_b, w1, b1, w2, b2)
    out, _ = run(in_maps, trace=False)
    return out



# revision 14
# speedup vs baseline: 1.7761x; 1.7761x over previous
"""ConfidenceGate Trainium2 kernel (8 NeuronCores, SPMD) — v3.

Problem recap (shapes hardcoded from the spec):
  x:      (4, 512, 256, 7, 7) f32
  prev_x: (4, 512, 256, 7, 7) f32
  match:  (4, 512, 513) f32
  + tiny proj/LN/MLP params.
Reference returns c[0] -> (512, 1): only batch 0 contributes to the output.

Strategy (v3 — PE-fused streaming):
  * Batch 0 only; data-parallel over M=512 rois: 8 cores x 64.
  * top1 = argmax(match[0,:,:512]) on host; prev rows pre-gathered per shard.
  * Pooling AND projection fused into ONE accumulating matmul chain on the
    Tensor engine: the stream is laid out as 98 blocks of [128 chan-half,
    128 rois(64 x | 64 v)] in fp8 (x*8, w*32 scaling keeps everything in
    e4m3 normal range; scales folded into the LN epilogue).  Weights
    ([128, 33] incl a negated-column-mean column for free centering) stay
    stationary; each block is one rhs stream -> psum[33, 128] accumulates
    pool+proj for x and v simultaneously.  proj_b rides in as a K=1
    rank-1 matmul.  No DVE pooling, no GpSimd in the hot path.
  * PE is pre-warmed with junk matmuls so the HAM clock-gate un-throttles
    (1.2 -> 2.4 GHz) before the real stream arrives.
  * psum[33,128] -> SBUF -> PE-transpose -> psum[128, 33] puts rois on
    partitions; LN/cosine/MLP run as a short per-partition-scalar chain
    on DVE with ACT only doing Ln / Sqrt / Sigmoid (tables primed early).
  * match stats ([64,512] fp16) run on DVE (second-max via is_equal mask),
    overlapped with the stream; 4 of 5 MLP features pre-accumulated.
"""

import sys

if "/opt/trn_rl_repo" not in sys.path:
    sys.path.insert(0, "/opt/trn_rl_repo")

import numpy as np
import ml_dtypes

B, M, N, C, G = 4, 512, 512, 256, 7
S = G * G                      # 49 spatial positions
PP, HH = 32, 32                # proj dim, MLP hidden
NCORES = 8
MS = M // NCORES               # 64 rois per core
CH = C // 2                    # 128 channels per half = contract dim
NBLK = 2 * S                   # 98 matmul blocks (h-major: b = h*49 + s)
BW = 2 * MS                    # 128 cols per block (64 x | 64 v)
NCHUNK = 7
BPC = NBLK // NCHUNK           # 14 blocks per chunk
SX, SW = 8.0, 32.0             # fp8 scales for data / weights
SEFF = S * SX * SW             # 12544 = total scale on psum values
EPS = 1e-9
LN_EPS = 1e-5
NEG = -60000.0                 # fp16-safe "-inf" for second-max masking
NPRE = 7                       # PE prewarm matmuls (N=512 each, ~0.43us cold)

F8 = ml_dtypes.float8_e4m3

# axf (f32) column layout, 64 partitions (one row per roi)
A_G = 0                        # ln_g / SEFF replicated (64, 32)
A_B = 32                       # ln_b replicated twice (64, 64) for [x|v] cols
A_W1 = 96                      # w1 block f at [96+32f : 128+32f), f=0..4
A_B1 = 256                     # b1 replicated (64, 32)
A_W2 = 288                     # w2[0] replicated (64, 32)
A_B2 = 320                     # b2 (64, 1)
A_COLS = 321

_CACHE = {}


def _build():
    import concourse.bacc as bacc
    import concourse.tile as tile
    import concourse.mybir as mybir

    dt = mybir.dt
    Alu = mybir.AluOpType
    Act = mybir.ActivationFunctionType
    Ax = mybir.AxisListType
    f32 = dt.float32
    f16 = dt.float16
    f8 = dt.float8e4

    nc = bacc.Bacc("TRN2", target_bir_lowering=False, debug=False)

    st_d = nc.dram_tensor("st", [CH, NBLK * BW], f8, kind="ExternalInput")
    mt_d = nc.dram_tensor("mt", [MS, N + 1], f16, kind="ExternalInput")
    wb_d = nc.dram_tensor("wb", [CH, 2 * (PP + 1)], f8, kind="ExternalInput")
    bo_d = nc.dram_tensor("bo", [1, (PP + 1) + BW], f16, kind="ExternalInput")
    id_d = nc.dram_tensor("idn", [PP + 1, PP + 1], f32, kind="ExternalInput")
    axf_d = nc.dram_tensor("axf", [MS, A_COLS], f32, kind="ExternalInput")
    out_d = nc.dram_tensor("out", [MS, 1], f32, kind="ExternalOutput")
    dbg_d = nc.dram_tensor("dbg", [1, 8], f32, kind="ExternalOutput")

    with tile.TileContext(nc) as tc, nc.allow_low_precision(
        reason="fp8 pool+proj stream + fp16 match stats; logit margin ~0.8 "
        "vs needed <0.79 flip, validated ~1e-3 logit err on host"
    ):
        with (
            tc.tile_pool(name="persist", bufs=1) as per,
            tc.tile_pool(name="chunks", bufs=1) as big,
            tc.tile_pool(name="scratch", bufs=1) as scr,
            tc.tile_pool(name="psum", bufs=1, space="PSUM") as psp,
        ):
            # ---- small loads on the scalar (ACT) HWDGE ring (wb/bo first:
            # they gate the matmul chain) ----
            wb = per.tile([CH, 2 * (PP + 1)], f8)
            nc.scalar.dma_start(out=wb[:], in_=wb_d[:])
            bo = per.tile([1, (PP + 1) + BW], f16)
            nc.scalar.dma_start(out=bo[:], in_=bo_d[:])
            mt = per.tile([MS, N + 1], f16)
            nc.scalar.dma_start(out=mt[:], in_=mt_d[:])
            idn = per.tile([PP + 1, PP + 1], f32)
            nc.scalar.dma_start(out=idn[:], in_=id_d[:])
            axf = per.tile([MS, A_COLS], f32)
            nc.scalar.dma_start(out=axf[:], in_=axf_d[:])

            # ---- big stream chunks: 4 on sync queue, 3 on gpsimd queue ----
            chunks = []
            for i in range(NCHUNK):
                t = big.tile([CH, BPC * BW], f8, tag=f"ch{i}", name=f"ch{i}")
                c0 = i * BPC * BW
                eng = nc.sync if i < 4 else nc.gpsimd
                eng.dma_start(out=t[:], in_=st_d[:, c0:c0 + BPC * BW])
                chunks.append(t)

            # ---- junk tile + PE prewarm (trip the HAM clock-gate while DMA
            # is still in flight; junk psum bank, no interaction) ----
            junk = per.tile([CH, 512], f16)
            nc.gpsimd.memset(junk[:], 0.25)
            e9 = per.tile([MS, 1], f32)
            nc.gpsimd.memset(e9[:], EPS)
            eln = per.tile([MS, 1], f32)
            nc.gpsimd.memset(eln[:], LN_EPS)
            jps = psp.tile([PP + 1, 512], f32, tag="jps", name="jps")
            for _ in range(NPRE):
                nc.tensor.matmul(jps[:], junk[:, 0:PP + 1], junk[:],
                                 start=True, stop=True, skip_group_check=True)

            # ---- dbg sentinel (kept: cheap, verifies DVE mult path) ----
            pr_in = scr.tile([1, 8], f32, tag="prin")
            nc.gpsimd.memset(pr_in[:], 30.0)
            prb = per.tile([1, 8], f32)
            nc.vector.tensor_scalar(prb[:], pr_in[:], 1.0, None, op0=Alu.mult)
            nc.scalar.dma_start(out=dbg_d[:], in_=prb[:])

            # ---- match stats on DVE (overlap the stream) ----
            real = mt[:, 0:N]
            pd = mt[:, N:N + 1]
            rmass = per.tile([MS, 1], f32)
            jr = scr.tile([MS, N], f16, tag="jr")
            nc.vector.tensor_scalar(jr[:], real, 1.0, 0.0, op0=Alu.mult,
                                    op1=Alu.add, accum_out=rmass[:])
            f_pmax = per.tile([MS, 1], f32)
            nc.vector.reduce_max(f_pmax[:], real, axis=Ax.X)
            eqm = scr.tile([MS, N], f16, tag="eqm")
            nc.vector.tensor_scalar(eqm[:], real, f_pmax[:], None,
                                    op0=Alu.is_equal)
            msk = scr.tile([MS, N], f16, tag="msk")
            nc.vector.scalar_tensor_tensor(msk[:], eqm[:], NEG, real,
                                           op0=Alu.mult, op1=Alu.add)
            m2 = per.tile([MS, 1], f32)
            nc.vector.reduce_max(m2[:], msk[:], axis=Ax.X)
            f_gap = per.tile([MS, 1], f32)
            nc.vector.tensor_tensor(f_gap[:], f_pmax[:], m2[:],
                                    op=Alu.subtract)
            # entropy feature is -ent = sum(real * ln(real+eps)) directly
            lnr = scr.tile([MS, N], f16, tag="lnr")
            nc.scalar.activation(lnr[:], real, Act.Ln, bias=e9[:])
            f_ent = per.tile([MS, 1], f32)
            je = scr.tile([MS, N], f16, tag="je")
            nc.vector.scalar_tensor_tensor(je[:], real, 1.0, lnr[:],
                                           op0=Alu.bypass, op1=Alu.mult,
                                           accum_out=f_ent[:])
            # prime the Sqrt table while the stream is still flowing
            dmy = scr.tile([1, 1], f32, tag="dmy")
            nc.gpsimd.memset(dmy[:], 1.0)
            pre = scr.tile([1, 1], f32, tag="pre")
            nc.scalar.activation(pre[:], dmy[:], Act.Sqrt)
            f_pd = per.tile([MS, 1], f32)
            nc.vector.tensor_scalar(f_pd[:], pd, -1.0, 1.0, op0=Alu.mult,
                                    op1=Alu.add)
            hr9 = per.tile([MS, 1], f32)
            nc.vector.tensor_scalar(hr9[:], rmass[:], EPS, None, op0=Alu.is_gt)
            hr6 = per.tile([MS, 1], f32)
            nc.vector.tensor_scalar(hr6[:], rmass[:], 1e-6, None,
                                    op0=Alu.is_gt)

            # ---- MLP hidden pre-accumulation over the 4 early features ----
            hA = scr.tile([MS, HH], f32, tag="hA")
            nc.vector.scalar_tensor_tensor(
                hA[:], axf[:, A_W1:A_W1 + HH], f_pd[:],
                axf[:, A_B1:A_B1 + HH], op0=Alu.mult, op1=Alu.add)
            hB = scr.tile([MS, HH], f32, tag="hB")
            nc.vector.scalar_tensor_tensor(
                hB[:], axf[:, A_W1 + HH:A_W1 + 2 * HH], f_pmax[:], hA[:],
                op0=Alu.mult, op1=Alu.add)
            hC = scr.tile([MS, HH], f32, tag="hC")
            nc.vector.scalar_tensor_tensor(
                hC[:], axf[:, A_W1 + 2 * HH:A_W1 + 3 * HH], f_gap[:], hB[:],
                op0=Alu.mult, op1=Alu.add)
            hD = per.tile([MS, HH], f32, tag="hD")
            nc.vector.scalar_tensor_tensor(
                hD[:], axf[:, A_W1 + 3 * HH:A_W1 + 4 * HH], f_ent[:], hC[:],
                op0=Alu.mult, op1=Alu.add)

            # ---- the fused pool+proj matmul chain ----
            ps1 = psp.tile([PP + 1, BW], f32, tag="ps1", name="ps1")
            # proj_b (pre-scaled, col-mean-negated) as a K=1 rank-1 update
            nc.tensor.matmul(ps1[:], bo[:, 0:PP + 1], bo[:, PP + 1:],
                             start=True, stop=False, skip_group_check=True)
            for b in range(NBLK):
                h = b // S
                ci, co = b // BPC, (b % BPC) * BW
                nc.tensor.matmul(
                    ps1[:], wb[:, h * (PP + 1):(h + 1) * (PP + 1)],
                    chunks[ci][:, co:co + BW],
                    start=False, stop=(b == NBLK - 1), skip_group_check=True)

            # ---- transpose psum -> [64 rois, 33 x | 33 v] (two transposes
            # so x and v share partitions as column groups; DVE ops cannot
            # mix SBUF base partitions) ----
            sb1 = per.tile([PP + 1, BW], f32)
            nc.vector.tensor_copy(sb1[:], ps1[:])
            ps2 = psp.tile([MS, 2 * (PP + 1)], f32, tag="ps2", name="ps2")
            nc.tensor.transpose(ps2[:, 0:PP + 1], sb1[:, 0:MS], idn[:])
            nc.tensor.transpose(ps2[:, PP + 1:], sb1[:, MS:BW], idn[:])

            # ---- layernorm (scale-folded) + cosine; w = 0 -> x, 1 -> v ----
            ctr = per.tile([MS, 2 * PP], f32)
            vs = per.tile([MS, 2], f32)
            for w in (0, 1):
                po = w * (PP + 1)
                nc.vector.tensor_scalar(ctr[:, w * PP:(w + 1) * PP],
                                        ps2[:, po:po + PP],
                                        ps2[:, po + PP:po + PP + 1],
                                        None, op0=Alu.add)
            jv = scr.tile([MS, 2 * PP], f32, tag="jv")
            for w in (0, 1):
                cs = ctr[:, w * PP:(w + 1) * PP]
                nc.vector.scalar_tensor_tensor(jv[:, w * PP:(w + 1) * PP],
                                               cs, 1.0, cs,
                                               op0=Alu.bypass, op1=Alu.mult,
                                               accum_out=vs[:, w:w + 1])
            sd = scr.tile([MS, 2], f32, tag="sd")
            nc.scalar.activation(sd[:], vs[:], Act.Sqrt,
                                 scale=1.0 / (PP * SEFF * SEFF), bias=eln[:])
            rstd = per.tile([MS, 2], f32)
            nc.vector.reciprocal(rstd[:], sd[:])
            gr = scr.tile([MS, 2 * PP], f32, tag="gr")
            for w in (0, 1):
                nc.vector.tensor_scalar(gr[:, w * PP:(w + 1) * PP],
                                        axf[:, A_G:A_G + PP],
                                        rstd[:, w:w + 1], None, op0=Alu.mult)
            yt = scr.tile([MS, 2 * PP], f32, tag="yt")
            nc.vector.tensor_tensor(yt[:], ctr[:], gr[:], op=Alu.mult)
            y = per.tile([MS, 2 * PP], f32)
            nc.vector.tensor_tensor(y[:], yt[:], axf[:, A_B:A_B + 2 * PP],
                                    op=Alu.add)
            ss = per.tile([MS, 2], f32)
            jn = scr.tile([MS, 2 * PP], f32, tag="jn")
            for w in (0, 1):
                ys = y[:, w * PP:(w + 1) * PP]
                nc.vector.scalar_tensor_tensor(jn[:, w * PP:(w + 1) * PP],
                                               ys, 1.0, ys,
                                               op0=Alu.bypass, op1=Alu.mult,
                                               accum_out=ss[:, w:w + 1])
            dot = per.tile([MS, 1], f32)
            jd = scr.tile([MS, PP], f32, tag="jd")
            nc.vector.scalar_tensor_tensor(jd[:], y[:, 0:PP], 1.0,
                                           y[:, PP:2 * PP], op0=Alu.bypass,
                                           op1=Alu.mult, accum_out=dot[:])
            s12 = scr.tile([MS, 1], f32, tag="s12")
            nc.vector.tensor_tensor(s12[:], ss[:, 0:1], ss[:, 1:2],
                                    op=Alu.mult)
            sq = scr.tile([MS, 1], f32, tag="sq")
            nc.scalar.activation(sq[:], s12[:], Act.Sqrt)
            rq = per.tile([MS, 1], f32)
            nc.vector.reciprocal(rq[:], sq[:])
            f_cos = per.tile([MS, 1], f32)
            nc.vector.scalar_tensor_tensor(f_cos[:], dot[:], rq[:], hr9[:],
                                           op0=Alu.mult, op1=Alu.mult)

            # ---- finish MLP ----
            hE = per.tile([MS, HH], f32)
            nc.vector.scalar_tensor_tensor(
                hE[:], axf[:, A_W1 + 4 * HH:A_W1 + 5 * HH], f_cos[:],
                hD[:], op0=Alu.mult, op1=Alu.add)
            hR = per.tile([MS, HH], f32)
            nc.vector.tensor_scalar(hR[:], hE[:], 0.0, None, op0=Alu.max)
            logit = per.tile([MS, 1], f32)
            jl = scr.tile([MS, HH], f32, tag="jl")
            nc.vector.scalar_tensor_tensor(jl[:], hR[:], 1.0,
                                           axf[:, A_W2:A_W2 + HH],
                                           op0=Alu.bypass, op1=Alu.mult,
                                           accum_out=logit[:])
            sg = per.tile([MS, 1], f32)
            nc.scalar.activation(sg[:], logit[:], Act.Sigmoid,
                                 bias=axf[:, A_B2:A_B2 + 1])
            gt = per.tile([MS, 1], f32)
            nc.vector.scalar_tensor_tensor(gt[:], sg[:], 0.999, hr6[:],
                                           op0=Alu.min, op1=Alu.mult)
            res = per.tile([MS, 1], f32)
            nc.vector.tensor_scalar(res[:], gt[:], 0.001, None, op0=Alu.max)
            nc.sync.dma_start(out=out_d[:], in_=res[:])

    nc.finalize()
    return nc


def _get_nc():
    if "nc" not in _CACHE:
        _CACHE["nc"] = _build()
    return _CACHE["nc"]


def make_in_maps(x, prev_x, match, proj_w, proj_b, ln_g, ln_b, w1, b1, w2, b2):
    f32 = np.float32
    f16 = np.float16
    x0 = np.asarray(x[0], dtype=f32).reshape(M, C, S)
    p0 = np.asarray(prev_x[0], dtype=f32).reshape(N, C, S)
    mt0 = np.ascontiguousarray(np.asarray(match[0], dtype=f32))
    real0 = mt0[:, :N]
    rm = real0.sum(axis=1)
    top1 = np.where(rm > EPS, np.argmax(real0, axis=1), 0)

    proj_w = np.asarray(proj_w, dtype=f32)   # (32, 256)
    proj_b = np.asarray(proj_b, dtype=f32)

    # stream: [core, 128 chan-half, 98 blocks (h-major) x 128 (64 x | 64 v)]
    def shard_blocks(rows):                  # (512, 256, 49) -> (8,2,49,128,64)
        return (rows.reshape(NCORES, MS, 2, CH, S)
                    .transpose(0, 2, 4, 3, 1))
    xt = shard_blocks(x0 * SX)
    vt = shard_blocks(p0[top1] * SX)
    comb = np.concatenate([xt, vt], axis=4)              # (8,2,49,128,128)
    stream = np.ascontiguousarray(
        comb.transpose(0, 3, 1, 2, 4).reshape(NCORES, CH, NBLK * BW)
    ).astype(F8)

    # weights: per half h, [128, 33]: cols 0:32 = 32*w[:, h*128+c].T,
    # col 32 = -32 * mean_p w  (the negated column-mean row for centering)
    wb = np.zeros((CH, 2 * (PP + 1)), dtype=f32)
    for h in (0, 1):
        blk = proj_w[:, h * CH:(h + 1) * CH].T * SW      # (128, 32)
        wb[:, h * (PP + 1):h * (PP + 1) + PP] = blk
        wb[:, h * (PP + 1) + PP] = -blk.mean(axis=1)
    wb = wb.astype(F8)

    bo = np.zeros((1, (PP + 1) + BW), dtype=f16)
    bo[0, :PP] = SEFF * proj_b
    bo[0, PP] = -SEFF * proj_b.mean()
    bo[0, PP + 1:] = 1.0

    idn = np.eye(PP + 1, dtype=f32)

    axf = np.zeros((MS, A_COLS), dtype=f32)
    axf[:, A_G:A_G + PP] = np.asarray(ln_g, dtype=f32) / SEFF
    axf[:, A_B:A_B + PP] = np.asarray(ln_b, dtype=f32)
    axf[:, A_B + PP:A_B + 2 * PP] = np.asarray(ln_b, dtype=f32)
    w1 = np.asarray(w1, dtype=f32)           # (32, 5)
    for f in range(5):
        axf[:, A_W1 + f * HH:A_W1 + (f + 1) * HH] = w1[:, f]
    axf[:, A_B1:A_B1 + HH] = np.asarray(b1, dtype=f32)
    axf[:, A_W2:A_W2 + HH] = np.asarray(w2, dtype=f32)[0]
    axf[:, A_B2] = np.asarray(b2, dtype=f32)[0]

    in_maps = []
    for i in range(NCORES):
        lo, hi = i * MS, (i + 1) * MS
        in_maps.append({
            "st": stream[i],
            "mt": np.ascontiguousarray(mt0[lo:hi]).astype(f16),
            "wb": wb, "bo": bo, "idn": idn, "axf": axf,
        })
    return in_maps


def run(in_maps, trace=False):
    from concourse.bass_utils import run_bass_kernel_spmd
    res = run_bass_kernel_spmd(_get_nc(), in_maps, list(range(NCORES)),
                               trace=trace)
    out = np.concatenate(
        [res.results[i]["out"].reshape(MS, 1) for i in range(NCORES)], axis=0)
    if trace:
        print("dbg sentinel (expect 30s):", res.results[0]["dbg"])
    return out.astype(np.float32), res


def kernel(x, prev_x, match, proj_w, proj_b, ln_g, ln_b, w1, b1, w2, b2):
    in_maps = make_in_maps(x, prev_x, match, proj_w, proj_b, ln_g, ln_b,
                           w1, b1, w2, b2)
    out, _ = run(in_maps, trace=False)
    return out


# revision 19
# speedup vs baseline: 2.1520x; 1.2116x over previous
"""ConfidenceGate Trainium2 kernel (8 NeuronCores, SPMD) — v4.

Problem recap (shapes hardcoded from the spec):
  x:      (4, 512, 256, 7, 7) f32
  prev_x: (4, 512, 256, 7, 7) f32
  match:  (4, 512, 513) f32
  + tiny proj/LN/MLP params.
Reference returns c[0] -> (512, 1): only batch 0 contributes to the output.

Strategy (v4 — PE-fused streaming, col-tiled, fast epilogue):
  * Batch 0 only; data-parallel over M=512 rois: 8 cores x 64.
  * top1 = argmax(match[0,:,:512]) on host; prev rows pre-gathered per shard.
  * Pooling AND projection fused into accumulating matmuls on the Tensor
    engine: 98 blocks of [128 chan-half, 128 rois(64 x | 64 v)] in fp8
    (x*8, w*32 scaling; scales folded into the epilogue).  Weights
    ([128, 33] incl a negated-column-mean column for free centering) are
    stationary; even/odd blocks go to PE col-groups (0,0)/(0,64) so two
    streams run concurrently on separate sub-arrays.  proj_b rides in as
    a K=1 f32 rank-1 matmul into group A.
  * PE pre-warmed with junk matmuls (HAM un-throttle) during the DMA fill;
    gpsimd memsets run BEFORE its SWDGE dma_starts (the SWDGE drain
    otherwise serializes them behind DMA completion).
  * ln_b == 0 fast path: per-roi rstd cancels exactly inside the
    normalized cosine, so the LN variance/sqrt/scale chain is skipped
    (u = g*(v-mu) feeds the cosine directly).  General path kept for
    nonzero ln_b.
  * result is PE-transposed to a [1, 64] row so the output DMA is one
    contiguous 256B descriptor (64 partition-strided 4B writes cost ~6us
    in completion-semaphore latency).
  * match stats ([64,512] fp16) on DVE, overlapped with the stream; 4 of
    5 MLP features pre-accumulated.  ACT tables: Ln early, Sqrt primed
    mid-stream, Sigmoid load hidden under the MLP DVE ops.
"""

import sys

if "/opt/trn_rl_repo" not in sys.path:
    sys.path.insert(0, "/opt/trn_rl_repo")

import numpy as np
import ml_dtypes

B, M, N, C, G = 4, 512, 512, 256, 7
S = G * G                      # 49 spatial positions
PP, HH = 32, 32                # proj dim, MLP hidden
NCORES = 8
MS = M // NCORES               # 64 rois per core
CH = C // 2                    # 128 channels per half = contract dim
NBLK = 2 * S                   # 98 matmul blocks (h-major: b = h*49 + s)
BW = 2 * MS                    # 128 cols per block (64 x | 64 v)
NCHUNK = 7
BPC = NBLK // NCHUNK           # 14 blocks per chunk
SX, SW = 8.0, 32.0             # fp8 scales for data / weights
SEFF = S * SX * SW             # 12544 = total scale on psum values
EPS = 1e-9
LN_EPS = 1e-5
NEG = -60000.0                 # fp16-safe "-inf" for second-max masking
NPRE = 6                       # PE prewarm matmuls (N=512 each, ~0.43us cold)
COLT = True                    # 2x PE column tiling (groups at col 0 / 64)

F8 = ml_dtypes.float8_e4m3

# axf (f32) column layout, 64 partitions (one row per roi)
A_G = 0                        # ln_g / SEFF replicated twice (64, 64) [x|v]
A_B = 64                       # ln_b replicated twice (64, 64) for [x|v]
A_W1 = 128                     # w1 block f at [128+32f : 160+32f), f=0..4
A_B1 = 288                     # b1 replicated (64, 32)
A_W2 = 320                     # w2[0] replicated (64, 32)
A_B2 = 352                     # b2 (64, 1)
A_BIAS = 353                   # SEFF*(proj_b | -mean) (row 0 only, 33)
A_ONES = 386                   # 1.0 x 128 (row 0 only)
A_ID64 = 514                   # 64x64 identity (rows 0..63)
A_COLS = 578

_CACHE = {}


def _build(fastpath):
    import concourse.bacc as bacc
    import concourse.tile as tile
    import concourse.mybir as mybir

    dt = mybir.dt
    Alu = mybir.AluOpType
    Act = mybir.ActivationFunctionType
    Ax = mybir.AxisListType
    f32 = dt.float32
    f16 = dt.float16
    f8 = dt.float8e4

    nc = bacc.Bacc("TRN2", target_bir_lowering=False, debug=False)

    st_d = nc.dram_tensor("st", [CH, NBLK * BW], f8, kind="ExternalInput")
    mt_d = nc.dram_tensor("mt", [MS, N + 1], f16, kind="ExternalInput")
    wb_d = nc.dram_tensor("wb", [CH, 2 * (PP + 1)], f8, kind="ExternalInput")
    id_d = nc.dram_tensor("idn", [PP + 1, PP + 1], f32, kind="ExternalInput")
    axf_d = nc.dram_tensor("axf", [MS, A_COLS], f32, kind="ExternalInput")
    out_d = nc.dram_tensor("out", [1, MS], f32, kind="ExternalOutput")
    dbg_d = nc.dram_tensor("dbg", [1, 8], f32, kind="ExternalOutput")

    NG = 2 if COLT else 1      # psum col groups

    with tile.TileContext(nc) as tc, nc.allow_low_precision(
        reason="fp8 pool+proj stream + fp16 match stats; logit margin ~0.79 "
        "vs ~1e-3 observed logit err (validated on host)"
    ):
        with (
            tc.tile_pool(name="persist", bufs=1) as per,
            tc.tile_pool(name="chunks", bufs=1) as big,
            tc.tile_pool(name="scratch", bufs=1) as scr,
            tc.tile_pool(name="psum", bufs=1, space="PSUM") as psp,
        ):
            # ---- small loads on the scalar (ACT) HWDGE ring; wb first
            # (gates matmuls), then axf (bias/identities), then mt ----
            wb = per.tile([CH, 2 * (PP + 1)], f8)
            nc.scalar.dma_start(out=wb[:], in_=wb_d[:])
            axf = per.tile([MS, A_COLS], f32)
            nc.scalar.dma_start(out=axf[:], in_=axf_d[:])
            mt = per.tile([MS, N + 1], f16)
            nc.scalar.dma_start(out=mt[:], in_=mt_d[:])
            # 33x33 identity replicated at partitions 0-32 and 64-96 for
            # the group-B transposes (rhs must live on the B row range)
            idn2 = per.tile([CH - 31, PP + 1], f32)
            nc.scalar.dma_start(out=idn2[0:PP + 1, :], in_=id_d[:])
            if COLT:
                nc.scalar.dma_start(out=idn2[MS:MS + PP + 1, :], in_=id_d[:])

            # ---- gpsimd: memsets FIRST (before its SWDGE dma_starts,
            # which drain-block anything queued after them) ----
            junk = per.tile([CH, 512], f16)
            nc.gpsimd.memset(junk[:], 0.25)
            e9 = per.tile([MS, 1], f32)
            nc.gpsimd.memset(e9[:], EPS)
            pr_in = scr.tile([1, 8], f32, tag="prin")
            nc.gpsimd.memset(pr_in[:], 30.0)
            if not fastpath:
                eln = per.tile([MS, 1], f32)
                nc.gpsimd.memset(eln[:], LN_EPS)

            # ---- big stream chunks: 4 on sync HWDGE, 3 on gpsimd SWDGE ----
            chunks = []
            for i in range(NCHUNK):
                t = big.tile([CH, BPC * BW], f8, tag=f"ch{i}", name=f"ch{i}")
                c0 = i * BPC * BW
                eng = nc.sync if i < 4 else nc.gpsimd
                eng.dma_start(out=t[:], in_=st_d[:, c0:c0 + BPC * BW])
                chunks.append(t)

            # ---- PE prewarm (HAM un-throttle during DMA fill) ----
            jps = psp.tile([PP + 1, 512], f32, tag="jps", name="jps")
            for _ in range(NPRE):
                nc.tensor.matmul(jps[:], junk[:, 0:PP + 1], junk[:],
                                 start=True, stop=True, skip_group_check=True)

            # ---- dbg sentinel ----
            prb = per.tile([1, 8], f32)
            nc.vector.tensor_scalar(prb[:], pr_in[:], 1.0, None, op0=Alu.mult)
            nc.scalar.dma_start(out=dbg_d[:], in_=prb[:])

            # ---- match stats on DVE (overlap the stream) ----
            real = mt[:, 0:N]
            pd = mt[:, N:N + 1]
            rmass = per.tile([MS, 1], f32)
            jr = scr.tile([MS, N], f16, tag="jr")
            nc.vector.tensor_scalar(jr[:], real, 1.0, 0.0, op0=Alu.mult,
                                    op1=Alu.add, accum_out=rmass[:])
            f_pmax = per.tile([MS, 1], f32)
            nc.vector.reduce_max(f_pmax[:], real, axis=Ax.X)
            eqm = scr.tile([MS, N], f16, tag="eqm")
            nc.vector.tensor_scalar(eqm[:], real, f_pmax[:], None,
                                    op0=Alu.is_equal)
            msk = scr.tile([MS, N], f16, tag="msk")
            nc.vector.scalar_tensor_tensor(msk[:], eqm[:], NEG, real,
                                           op0=Alu.mult, op1=Alu.add)
            m2 = per.tile([MS, 1], f32)
            nc.vector.reduce_max(m2[:], msk[:], axis=Ax.X)
            f_gap = per.tile([MS, 1], f32)
            nc.vector.tensor_tensor(f_gap[:], f_pmax[:], m2[:],
                                    op=Alu.subtract)
            # entropy feature is -ent = sum(real * ln(real+eps)) directly
            lnr = scr.tile([MS, N], f16, tag="lnr")
            nc.scalar.activation(lnr[:], real, Act.Ln, bias=e9[:])
            f_ent = per.tile([MS, 1], f32)
            je = scr.tile([MS, N], f16, tag="je")
            nc.vector.scalar_tensor_tensor(je[:], real, 1.0, lnr[:],
                                           op0=Alu.bypass, op1=Alu.mult,
                                           accum_out=f_ent[:])
            # prime the Sqrt table while the stream is still flowing
            dmy = scr.tile([1, 1], f32, tag="dmy")
            nc.gpsimd.memset(dmy[:], 1.0)
            pre = scr.tile([1, 1], f32, tag="pre")
            nc.scalar.activation(pre[:], dmy[:], Act.Sqrt)
            f_pd = per.tile([MS, 1], f32)
            nc.vector.tensor_scalar(f_pd[:], pd, -1.0, 1.0, op0=Alu.mult,
                                    op1=Alu.add)
            hr9 = per.tile([MS, 1], f32)
            nc.vector.tensor_scalar(hr9[:], rmass[:], EPS, None, op0=Alu.is_gt)
            hr6 = per.tile([MS, 1], f32)
            nc.vector.tensor_scalar(hr6[:], rmass[:], 1e-6, None,
                                    op0=Alu.is_gt)

            # ---- MLP hidden pre-accumulation over the 4 early features ----
            hA = scr.tile([MS, HH], f32, tag="hA")
            nc.vector.scalar_tensor_tensor(
                hA[:], axf[:, A_W1:A_W1 + HH], f_pd[:],
                axf[:, A_B1:A_B1 + HH], op0=Alu.mult, op1=Alu.add)
            hB = scr.tile([MS, HH], f32, tag="hB")
            nc.vector.scalar_tensor_tensor(
                hB[:], axf[:, A_W1 + HH:A_W1 + 2 * HH], f_pmax[:], hA[:],
                op0=Alu.mult, op1=Alu.add)
            hC = scr.tile([MS, HH], f32, tag="hC")
            nc.vector.scalar_tensor_tensor(
                hC[:], axf[:, A_W1 + 2 * HH:A_W1 + 3 * HH], f_gap[:], hB[:],
                op0=Alu.mult, op1=Alu.add)
            hD = per.tile([MS, HH], f32, tag="hD")
            nc.vector.scalar_tensor_tensor(
                hD[:], axf[:, A_W1 + 3 * HH:A_W1 + 4 * HH], f_ent[:], hC[:],
                op0=Alu.mult, op1=Alu.add)

            # ---- the fused pool+proj matmul chain ----
            # ps1 partitions [0:33] = group A, [64:97] = group B
            ps1 = psp.tile([CH, BW], f32, tag="ps1", name="ps1")
            # proj_b as a K=1 f32 rank-1 update into group A
            nc.tensor.matmul(ps1[0:PP + 1, :],
                             axf[0:1, A_BIAS:A_BIAS + PP + 1],
                             axf[0:1, A_ONES:A_ONES + BW],
                             start=True, stop=False, skip_group_check=True,
                             tile_position=(0, 0) if COLT else None)
            started_b = False
            for b in range(NBLK):
                h = b // S
                ci, co = b // BPC, (b % BPC) * BW
                grp = (b % 2) if COLT else 0
                po = 0 if grp == 0 else MS
                st_flag = False
                if grp == 1 and not started_b:
                    st_flag, started_b = True, True
                stop_flag = (b >= NBLK - 2) if COLT else (b == NBLK - 1)
                nc.tensor.matmul(
                    ps1[po:po + PP + 1, :],
                    wb[:, h * (PP + 1):(h + 1) * (PP + 1)],
                    chunks[ci][:, co:co + BW],
                    start=st_flag, stop=stop_flag,
                    skip_group_check=True,
                    tile_position=(0, po) if COLT else None)

            # ---- psum -> SBUF -> PE-transpose -> ps2[64, NG*2*33] ----
            sb1 = per.tile([CH - 31, BW], f32)
            nc.vector.tensor_copy(sb1[0:PP + 1, :], ps1[0:PP + 1, :])
            if COLT:
                nc.vector.tensor_copy(sb1[MS:MS + PP + 1, :],
                                      ps1[MS:MS + PP + 1, :])
            ps2 = psp.tile([MS, 2 * (PP + 1)], f32, tag="ps2", name="ps2")
            W33 = PP + 1
            # group-B transposes ACCUMULATE onto group A's region, merging
            # the col-tiled halves for free on the PE
            nc.tensor.matmul(ps2[:, 0:W33], sb1[0:W33, 0:MS],
                             idn2[0:W33, :], is_transpose=True,
                             start=True, stop=not COLT,
                             skip_group_check=True, tile_position=(0, 0))
            nc.tensor.matmul(ps2[:, W33:2 * W33], sb1[0:W33, MS:BW],
                             idn2[0:W33, :], is_transpose=True,
                             start=True, stop=not COLT,
                             skip_group_check=True, tile_position=(0, 0))
            if COLT:
                nc.tensor.matmul(ps2[:, 0:W33], sb1[MS:MS + W33, 0:MS],
                                 idn2[MS:MS + W33, :], is_transpose=True,
                                 start=False, stop=True,
                                 skip_group_check=True,
                                 tile_position=(MS, 0))
                nc.tensor.matmul(ps2[:, W33:2 * W33],
                                 sb1[MS:MS + W33, MS:BW],
                                 idn2[MS:MS + W33, :], is_transpose=True,
                                 start=False, stop=True,
                                 skip_group_check=True,
                                 tile_position=(MS, 0))

            # ---- center:  src cols = [v | -mu] per roi, per w-group ----
            src = ps2
            ctr = per.tile([MS, 2 * PP], f32)
            for w in (0, 1):
                po = w * W33
                nc.vector.tensor_scalar(ctr[:, w * PP:(w + 1) * PP],
                                        src[:, po:po + PP],
                                        src[:, po + PP:po + PP + 1],
                                        None, op0=Alu.add)

            if fastpath:
                # ln_b == 0: rstd cancels in the normalized cosine, so
                # cos = <g*ctr_x, g*ctr_v> / (|g*ctr_x| |g*ctr_v|)
                u = per.tile([MS, 2 * PP], f32)
                nc.vector.tensor_tensor(u[:], ctr[:],
                                        axf[:, A_G:A_G + 2 * PP],
                                        op=Alu.mult)
                ss = per.tile([MS, 2], f32)
                jn = scr.tile([MS, 2 * PP], f32, tag="jn")
                for w in (0, 1):
                    us = u[:, w * PP:(w + 1) * PP]
                    nc.vector.scalar_tensor_tensor(
                        jn[:, w * PP:(w + 1) * PP], us, 1.0, us,
                        op0=Alu.bypass, op1=Alu.mult,
                        accum_out=ss[:, w:w + 1])
                dot = per.tile([MS, 1], f32)
                jd = scr.tile([MS, PP], f32, tag="jd")
                nc.vector.scalar_tensor_tensor(jd[:], u[:, 0:PP], 1.0,
                                               u[:, PP:2 * PP],
                                               op0=Alu.bypass, op1=Alu.mult,
                                               accum_out=dot[:])
            else:
                vs = per.tile([MS, 2], f32)
                jv = scr.tile([MS, 2 * PP], f32, tag="jv")
                for w in (0, 1):
                    cs = ctr[:, w * PP:(w + 1) * PP]
                    nc.vector.scalar_tensor_tensor(
                        jv[:, w * PP:(w + 1) * PP], cs, 1.0, cs,
                        op0=Alu.bypass, op1=Alu.mult,
                        accum_out=vs[:, w:w + 1])
                sd = scr.tile([MS, 2], f32, tag="sd")
                nc.scalar.activation(sd[:], vs[:], Act.Sqrt,
                                     scale=1.0 / (PP * SEFF * SEFF),
                                     bias=eln[:])
                rstd = per.tile([MS, 2], f32)
                nc.vector.reciprocal(rstd[:], sd[:])
                gr = scr.tile([MS, 2 * PP], f32, tag="gr")
                for w in (0, 1):
                    nc.vector.tensor_scalar(gr[:, w * PP:(w + 1) * PP],
                                            axf[:, A_G + w * PP:
                                                A_G + (w + 1) * PP],
                                            rstd[:, w:w + 1], None,
                                            op0=Alu.mult)
                yt = scr.tile([MS, 2 * PP], f32, tag="yt")
                nc.vector.tensor_tensor(yt[:], ctr[:], gr[:], op=Alu.mult)
                u = per.tile([MS, 2 * PP], f32)
                nc.vector.tensor_tensor(u[:], yt[:],
                                        axf[:, A_B:A_B + 2 * PP], op=Alu.add)
                ss = per.tile([MS, 2], f32)
                jn = scr.tile([MS, 2 * PP], f32, tag="jn")
                for w in (0, 1):
                    ys = u[:, w * PP:(w + 1) * PP]
                    nc.vector.scalar_tensor_tensor(
                        jn[:, w * PP:(w + 1) * PP], ys, 1.0, ys,
                        op0=Alu.bypass, op1=Alu.mult,
                        accum_out=ss[:, w:w + 1])
                dot = per.tile([MS, 1], f32)
                jd = scr.tile([MS, PP], f32, tag="jd")
                nc.vector.scalar_tensor_tensor(jd[:], u[:, 0:PP], 1.0,
                                               u[:, PP:2 * PP],
                                               op0=Alu.bypass, op1=Alu.mult,
                                               accum_out=dot[:])

            s12 = scr.tile([MS, 1], f32, tag="s12")
            nc.vector.tensor_tensor(s12[:], ss[:, 0:1], ss[:, 1:2],
                                    op=Alu.mult)
            sq = scr.tile([MS, 1], f32, tag="sq")
            nc.scalar.activation(sq[:], s12[:], Act.Sqrt)
            rq = per.tile([MS, 1], f32)
            nc.vector.reciprocal(rq[:], sq[:])
            f_cos = per.tile([MS, 1], f32)
            nc.vector.scalar_tensor_tensor(f_cos[:], dot[:], rq[:], hr9[:],
                                           op0=Alu.mult, op1=Alu.mult)

            # ---- finish MLP ----
            hE = per.tile([MS, HH], f32)
            nc.vector.scalar_tensor_tensor(
                hE[:], axf[:, A_W1 + 4 * HH:A_W1 + 5 * HH], f_cos[:],
                hD[:], op0=Alu.mult, op1=Alu.add)
            hR = per.tile([MS, HH], f32)
            nc.vector.tensor_scalar(hR[:], hE[:], 0.0, None, op0=Alu.max)
            logit = per.tile([MS, 1], f32)
            jl = scr.tile([MS, HH], f32, tag="jl")
            nc.vector.scalar_tensor_tensor(jl[:], hR[:], 1.0,
                                           axf[:, A_W2:A_W2 + HH],
                                           op0=Alu.bypass, op1=Alu.mult,
                                           accum_out=logit[:])
            sg = per.tile([MS, 1], f32)
            nc.scalar.activation(sg[:], logit[:], Act.Sigmoid,
                                 bias=axf[:, A_B2:A_B2 + 1])
            gt = per.tile([MS, 1], f32)
            nc.vector.scalar_tensor_tensor(gt[:], sg[:], 0.999, hr6[:],
                                           op0=Alu.min, op1=Alu.mult)
            res = per.tile([MS, 1], f32)
            nc.vector.tensor_scalar(res[:], gt[:], 0.001, None, op0=Alu.max)

            # ---- transpose result to a [1, 64] row -> single contiguous
            # 256B output DMA (partition-strided 4B writes stall the
            # completion semaphore ~6us) ----
            pout = psp.tile([1, MS], f32, tag="pout", name="pout")
            nc.tensor.matmul(pout[:], res[:],
                             axf[:, A_ID64:A_ID64 + MS],
                             start=True, stop=True, skip_group_check=True)
            rrow = per.tile([1, MS], f32)
            nc.vector.tensor_copy(rrow[:], pout[:])
            nc.sync.dma_start(out=out_d[:], in_=rrow[:])

    nc.finalize()
    return nc


def _get_nc(fastpath):
    key = ("nc", fastpath)
    if key not in _CACHE:
        _CACHE[key] = _build(fastpath)
    return _CACHE[key]


def make_in_maps(x, prev_x, match, proj_w, proj_b, ln_g, ln_b, w1, b1, w2, b2):
    f32 = np.float32
    f16 = np.float16
    x0 = np.asarray(x[0], dtype=f32).reshape(M, C, S)
    p0 = np.asarray(prev_x[0], dtype=f32).reshape(N, C, S)
    mt0 = np.ascontiguousarray(np.asarray(match[0], dtype=f32))
    real0 = mt0[:, :N]
    rm = real0.sum(axis=1)
    top1 = np.where(rm > EPS, np.argmax(real0, axis=1), 0)

    proj_w = np.asarray(proj_w, dtype=f32)   # (32, 256)
    proj_b = np.asarray(proj_b, dtype=f32)

    # stream: [core, 128 chan-half, 98 blocks (h-major) x 128 (64 x | 64 v)]
    def shard_blocks(rows):                  # (512, 256, 49) -> (8,2,49,128,64)
        return (rows.reshape(NCORES, MS, 2, CH, S)
                    .transpose(0, 2, 4, 3, 1))
    xt = shard_blocks(x0 * SX)
    vt = shard_blocks(p0[top1] * SX)
    comb = np.concatenate([xt, vt], axis=4)              # (8,2,49,128,128)
    stream = np.ascontiguousarray(
        comb.transpose(0, 3, 1, 2, 4).reshape(NCORES, CH, NBLK * BW)
    ).astype(F8)

    # weights: per half h, [128, 33]: cols 0:32 = 32*w[:, h*128+c].T,
    # col 32 = -32 * mean_p w  (negated column-mean row for centering)
    wb = np.zeros((CH, 2 * (PP + 1)), dtype=f32)
    for h in (0, 1):
        blk = proj_w[:, h * CH:(h + 1) * CH].T * SW      # (128, 32)
        wb[:, h * (PP + 1):h * (PP + 1) + PP] = blk
        wb[:, h * (PP + 1) + PP] = -blk.mean(axis=1)
    wb = wb.astype(F8)

    idn = np.eye(PP + 1, dtype=f32)

    axf = np.zeros((MS, A_COLS), dtype=f32)
    ln_g = np.asarray(ln_g, dtype=f32)
    ln_b = np.asarray(ln_b, dtype=f32)
    fastpath = bool(np.all(ln_b == 0.0))
    # fastpath cosine is scale-invariant -> raw ln_g; general path folds
    # the stream scale into g (y = ctrS * rstd_true * g/SEFF)
    gfill = ln_g if fastpath else ln_g / SEFF
    axf[:, A_G:A_G + PP] = gfill
    axf[:, A_G + PP:A_G + 2 * PP] = gfill
    axf[:, A_B:A_B + PP] = ln_b
    axf[:, A_B + PP:A_B + 2 * PP] = ln_b
    w1 = np.asarray(w1, dtype=f32)           # (32, 5)
    for f in range(5):
        axf[:, A_W1 + f * HH:A_W1 + (f + 1) * HH] = w1[:, f]
    axf[:, A_B1:A_B1 + HH] = np.asarray(b1, dtype=f32)
    axf[:, A_W2:A_W2 + HH] = np.asarray(w2, dtype=f32)[0]
    axf[:, A_B2] = np.asarray(b2, dtype=f32)[0]
    axf[0, A_BIAS:A_BIAS + PP] = SEFF * proj_b
    axf[0, A_BIAS + PP] = -SEFF * proj_b.mean()
    axf[0, A_ONES:A_ONES + BW] = 1.0
    axf[:, A_ID64:A_ID64 + MS] = np.eye(MS, dtype=f32)

    in_maps = []
    for i in range(NCORES):
        lo, hi = i * MS, (i + 1) * MS
        in_maps.append({
            "st": stream[i],
            "mt": np.ascontiguousarray(mt0[lo:hi]).astype(f16),
            "wb": wb, "idn": idn, "axf": axf,
        })
    return in_maps, fastpath


def run(in_maps, fastpath=True, trace=False):
    from concourse.bass_utils import run_bass_kernel_spmd
    res = run_bass_kernel_spmd(_get_nc(fastpath), in_maps,
                               list(range(NCORES)), trace=trace)
    out = np.concatenate(
        [res.results[i]["out"].reshape(MS, 1) for i in range(NCORES)], axis=0)
    if trace:
        print("dbg sentinel (expect 30s):", res.results[0]["dbg"])
    return out.astype(np.float32), res


def kernel(x, prev_x, match, proj_w, proj_b, ln_g, ln_b, w1, b1, w2, b2):
    in_maps, fastpath = make_in_maps(x, prev_x, match, proj_w, proj_b,
                                     ln_g, ln_b, w1, b1, w2, b2)
    out, _ = run(in_maps, fastpath=fastpath, trace=False)
    return out
